# revision 6
# baseline (speedup 1.0000x reference)
"""Trainium2 Bass kernel for a LoRA-MoE layer (gate top-2 softmax routing +
dense base linear + per-expert low-rank adapters), SPMD across 8 NeuronCores.

Math (per token t):
    logits = x @ gate_w.T                      # [E]
    top-2 softmax over logits -> dense w[E] (0 for non-selected)
    out = x @ base_w.T + base_b
        + SCALING * sum_e w[e] * (x @ lora_A[e].T) @ lora_B[e].T

Key identities:
  * w folded into rank-space activations: lora_out = (low * w_rep) @ B_all.T
    with low = x @ A_all.T (A_all: [E*R, D]) -> whole MoE-LoRA is two dense
    matmuls + tiny gating vector math.
  * top-2 softmax via sigmoid: w_e = [l_e >= m2] * sigmoid(2*l_e - m1 - m2)
    (for the top-1 expert this is sigmoid(l1-l2), for top-2 sigmoid(l2-l1)).

Sharding: 8-way data parallel over tokens (T=512 tokens per core), base W
replicated and streamed.  This halves the x-load + phase-A serial head vs a
token x out-feature split; W streaming needs only ~150 GB/s per core.

Performance structure:
  * all matmul operands bf16 (host cast, free) -> PE rate unchanged, HBM
    bytes halved.
  * DMA order: adapters + x chunks first on both rings, W strictly behind x.
  * ~16 dummy matmuls at t~4us keep the PE HAM clock-gate warm before the
    first x chunk lands.
  * single shared 8-slot PSUM pool; out-tile k-loops run back-to-back while
    the gating vector chain (DVE/ACT/GPSIMD) hides behind them; each
    out-tile's B-adapter "stop" matmul is deferred two groups.
"""

import numpy as np
import ml_dtypes

import concourse.bass as bass
import concourse.bass_isa as bass_isa
import concourse.mybir as mybir
import concourse.tile as tile
from concourse import bacc
from concourse.bass_utils import run_bass_kernel_spmd

F32 = mybir.dt.float32
BF16 = mybir.dt.bfloat16
NPBF16 = ml_dtypes.bfloat16

# Problem constants
B, S, D, O = 2, 2048, 4096, 4096
E, R = 8, 16
ER = E * R  # 128
SCALING = 32.0 / 16.0

# Sharding: 8 token groups, W replicated
N_CORES = 8
TG = 8
T = (B * S) // TG       # 512 tokens per core
KT = D // 128           # 32 contraction tiles
OTN = O // 128          # 32 out tiles per core
XC = 8                  # x DMA chunks
KPC = KT // XC          # 4 k-tiles per chunk
NWARM = 8              # PE warm-up matmuls


def build_body(nc, tc, tensors):
    xT, wT, aT, gT, bT, bias2, Rm, out = tensors
    OP = mybir.AluOpType
    ACT = mybir.ActivationFunctionType

    with (
        tc.tile_pool(name="xp", bufs=XC) as xp,
        tc.tile_pool(name="wp", bufs=8) as wp,
        tc.tile_pool(name="cst", bufs=1) as cst,
        tc.tile_pool(name="gw", bufs=1) as gw,
        tc.tile_pool(name="outp", bufs=2) as outp,
        tc.tile_pool(name="ps", bufs=8, space="PSUM") as ps,
    ):
        # ---- DMA program.  The gate/lora-A/x slices for each group of 4
        #      k-tiles are fused host-side into one contiguous per-partition
        #      payload (~5KB lines, no small packets); chunks alternate
        #      across the sync and scalar rings.  W queues strictly behind
        #      the chunks on the sync ring; bias/Rm/bT slot in between the
        #      first W tiles (needed only from ~40us). ----
        GW, AW, XW = KPC * E, KPC * ER, KPC * T
        CW = GW + AW + XW
        x_tiles = []
        for c in range(XC):
            eng = nc.sync if c % 2 == 0 else nc.scalar
            xc_t = xp.tile([128, CW], BF16, tag="x", name=f"x{c}")
            eng.dma_start(out=xc_t[:], in_=xT[:, c, :])
            x_tiles.append(xc_t)

        w_tiles = []
        for ot in range(OTN):
            wv = wp.tile([128, KT * 128], BF16, tag="w", name=f"w{ot}")
            nc.sync.dma_start(out=wv[:], in_=wT[:, ot, :])
            w_tiles.append(wv)
            if ot == 1:
                bias_sb = cst.tile([128, OTN], F32)
                nc.sync.dma_start(out=bias_sb[:], in_=bias2[:])
                Rm_sb = cst.tile([E, ER], BF16)
                nc.sync.dma_start(out=Rm_sb[:], in_=Rm[:])
            if ot == 3:
                bT_sb = cst.tile([ER, O], BF16)
                nc.sync.dma_start(out=bT_sb[:], in_=bT[:])

        def gs(k):
            """gate_w.T slice [128, E] for k-tile k."""
            return x_tiles[k // KPC][:, (k % KPC) * E:(k % KPC + 1) * E]

        def as_(k):
            """lora_A.T slice [128, ER] for k-tile k."""
            return x_tiles[k // KPC][:, GW + (k % KPC) * ER:
                                     GW + (k % KPC + 1) * ER]

        def xs(k):
            """x.T slice [128, T] for k-tile k."""
            return x_tiles[k // KPC][:, GW + AW + (k % KPC) * T:
                                     GW + AW + (k % KPC + 1) * T]

        # ---- PE warm-up: dummy matmuls on zeros so the HAM clock gate is
        #      already at 8/8 when the first x chunk lands.  They write the
        #      gate PSUM bank; the real k=0 matmul's start=True wipes them. ----
        warm = cst.tile([128, T], BF16)
        nc.vector.memset(warm[:], 0.0)

        gate_ps = ps.tile([E, T], F32, tag="ps", name="gateps")
        low_ps = ps.tile([ER, T], F32, tag="ps", name="lowps")
        for i in range(NWARM):
            nc.tensor.matmul(gate_ps[:], lhsT=warm[:, :E], rhs=warm[:],
                             start=True, stop=True, skip_group_check=True)

        # ---- phase A: low.T = A_all.T^T @ x.T ; gate.T = g^T @ x.T ----
        for k in range(KT):
            nc.tensor.matmul(gate_ps[:], lhsT=gs(k),
                             rhs=xs(k), start=(k == 0), stop=(k == KT - 1),
                             skip_group_check=(k == 0))
            nc.tensor.matmul(low_ps[:], lhsT=as_(k),
                             rhs=xs(k), start=(k == 0), stop=(k == KT - 1))

        # ---- gating math in [E, t] layout (DVE/ACT/GPSIMD; overlaps the
        #      first base-W matmul groups on the PE) ----
        # w_e = [l_e >= m2] * sigmoid(2*l_e - m1 - m2) * SCALING
        lowT_sb = cst.tile([ER, T], BF16, tag="lowT")
        g_sb = gw.tile([E, T], F32, tag="g")
        nc.scalar.copy(g_sb[:], gate_ps[:])
        m1 = gw.tile([E, T], F32, tag="m1")
        nc.gpsimd.partition_all_reduce(m1[:], g_sb[:], channels=E,
                                       reduce_op=bass_isa.ReduceOp.max)
        eq = gw.tile([E, T], F32, tag="eq")
        nc.vector.tensor_tensor(eq[:], g_sb[:], m1[:], op=OP.is_equal)
        gm = gw.tile([E, T], F32, tag="gm")
        nc.vector.scalar_tensor_tensor(gm[:], in0=eq[:], scalar=-1e30,
                                       in1=g_sb[:], op0=OP.mult, op1=OP.add)
        m2 = gw.tile([E, T], F32, tag="m2")
        nc.gpsimd.partition_all_reduce(m2[:], gm[:], channels=E,
                                       reduce_op=bass_isa.ReduceOp.max)
        t1 = gw.tile([E, T], F32, tag="t1")
        nc.vector.tensor_tensor(t1[:], m1[:], m2[:], op=OP.add)
        s = gw.tile([E, T], F32, tag="s")
        nc.vector.scalar_tensor_tensor(s[:], in0=g_sb[:], scalar=2.0,
                                       in1=t1[:], op0=OP.mult, op1=OP.subtract)
        sig = gw.tile([E, T], F32, tag="sig")
        nc.scalar.activation(sig[:], s[:], ACT.Sigmoid)
        mask = gw.tile([E, T], F32, tag="mask")
        nc.vector.tensor_tensor(mask[:], g_sb[:], m2[:], op=OP.is_ge)
        wsc = gw.tile([E, T], BF16, tag="wsc")
        nc.vector.scalar_tensor_tensor(wsc[:], in0=sig[:], scalar=SCALING,
                                       in1=mask[:], op0=OP.mult, op1=OP.mult)

        # ---- phase B: out.T[ot] = sum_k W[ot,k]^T @ x.T (+ B^T @ low_w.T) ----
        def w_group(ot, pb):
            for k in range(KT):
                nc.tensor.matmul(pb[:], lhsT=w_tiles[ot][:, k * 128:(k + 1) * 128],
                                 rhs=xs(k), start=(k == 0), stop=False)

        def stop_group(ot, pb):
            nc.tensor.matmul(pb[:], lhsT=bT_sb[:, ot * 128:(ot + 1) * 128],
                             rhs=lowT_sb[:], start=False, stop=True)
            o_sb = outp.tile([128, T], BF16, tag="o", name=f"o{ot}")
            nc.vector.tensor_scalar(o_sb[:], pb[:],
                                    scalar1=bias_sb[:, ot:ot + 1],
                                    scalar2=None, op0=OP.add)
            nc.gpsimd.dma_start(out=out[:, ot, :], in_=o_sb[:])

        pbs = []
        for ot in range(OTN):
            pb = ps.tile([128, T], F32, tag="ps", name=f"pb{ot}")
            pbs.append(pb)
            w_group(ot, pb)
            if ot == 1:
                # replicate each expert weight over its 16 ranks via a tiny
                # matmul, then fold into the rank-space activations.  Emitted
                # after ot1's W MMs so the PE never waits on the gating chain.
                wrep_ps = ps.tile([ER, T], F32, tag="ps", name="wrep")
                nc.tensor.matmul(wrep_ps[:], lhsT=Rm_sb[:], rhs=wsc[:],
                                 start=True, stop=True)
                wrep_sb = gw.tile([ER, T], F32, tag="wrepsb")
                nc.scalar.copy(wrep_sb[:], wrep_ps[:])
                # low_w.T = low.T * w_rep (DVE: one PSUM operand only)
                nc.vector.tensor_tensor(lowT_sb[:], low_ps[:], wrep_sb[:],
                                        op=OP.mult)
            if ot >= 2:
                stop_group(ot - 2, pbs[ot - 2])
        stop_group(OTN - 2, pbs[OTN - 2])
        stop_group(OTN - 1, pbs[OTN - 1])


def build_module(debug=False):
    nc = bacc.Bacc("TRN2", target_bir_lowering=False, debug=debug)
    CW = KPC * (E + ER + T)
    xT = nc.dram_tensor("xT", [128, XC, CW], BF16, kind="ExternalInput")
    wT = nc.dram_tensor("wT", [128, OTN, KT * 128], BF16, kind="ExternalInput")
    aT = None
    gT = None
    bT = nc.dram_tensor("bT", [ER, O], BF16, kind="ExternalInput")
    bias2 = nc.dram_tensor("bias2", [128, OTN], F32, kind="ExternalInput")
    Rm = nc.dram_tensor("Rm", [E, ER], BF16, kind="ExternalInput")
    out = nc.dram_tensor("out", [128, OTN, T], BF16, kind="ExternalOutput")
    with tile.TileContext(nc) as tc:
        build_body(nc, tc, (xT, wT, aT, gT, bT, bias2, Rm, out))
    nc.compile()
    return nc


def shard_inputs(x, gate_w, base_w, base_b, lora_A, lora_B):
    """FULL inputs -> list of 8 per-core input maps (host-side, free)."""
    x = np.asarray(x, dtype=np.float32)
    gate_w = np.asarray(gate_w, dtype=np.float32)
    base_w = np.asarray(base_w, dtype=np.float32)
    base_b = np.asarray(base_b, dtype=np.float32)
    lora_A = np.asarray(lora_A, dtype=np.float32)
    lora_B = np.asarray(lora_B, dtype=np.float32)

    xf = x.reshape(B * S, D)
    # replicated tensors; gate/lora-A slices fused into each x chunk payload
    gp = gate_w.T.reshape(XC, KPC, 128, E).transpose(2, 0, 1, 3)  # [128,XC,KPC,E]
    A_flat = lora_A.reshape(ER, D)
    ap = A_flat.T.reshape(XC, KPC, 128, ER).transpose(2, 0, 1, 3)
    B_flat = lora_B.transpose(0, 2, 1).reshape(ER, O)   # [er, o]
    bT = np.ascontiguousarray(B_flat).astype(NPBF16)
    Rm = np.repeat(np.eye(E, dtype=np.float32), R, axis=1).astype(NPBF16)
    wT = np.ascontiguousarray(
        base_w.reshape(OTN, 128, KT, 128).transpose(3, 0, 2, 1)
        .reshape(128, OTN, KT * 128)).astype(NPBF16)
    bias2 = np.ascontiguousarray(base_b.reshape(OTN, 128).T)
    gac = np.concatenate([
        gp.reshape(128, XC, KPC * E),
        ap.reshape(128, XC, KPC * ER)], axis=2)          # [128, XC, GW+AW]

    in_maps = []
    for c in range(N_CORES):
        x_c = xf[c * T:(c + 1) * T]                         # [T, D]
        xp_ = x_c.T.reshape(XC, KPC, 128, T).transpose(2, 0, 1, 3)
        xT = np.ascontiguousarray(np.concatenate(
            [gac, xp_.reshape(128, XC, KPC * T)], axis=2)).astype(NPBF16)
        in_maps.append({"xT": xT, "wT": wT,
                        "bT": bT, "bias2": bias2, "Rm": Rm})
    return in_maps


def gather_outputs(results):
    """list of 8 per-core result maps -> FULL output [B, S, O]."""
    full = np.empty((B * S, O), dtype=np.float32)
    for c in range(N_CORES):
        oc = np.asarray(results[c]["out"], dtype=np.float32)  # [128, OTN, T]
        full[c * T:(c + 1) * T, :] = oc.transpose(2, 1, 0).reshape(T, O)
    return full.reshape(B, S, O)


_NC_CACHE = {}


def _get_module():
    if "nc" not in _NC_CACHE:
        _NC_CACHE["nc"] = build_module()
    return _NC_CACHE["nc"]


def run_sharded(in_maps, **run_kwargs):
    nc = _get_module()
    return run_bass_kernel_spmd(nc, in_maps, list(range(N_CORES)), **run_kwargs)


def kernel(x, gate_w, base_w, base_b, lora_A, lora_B):
    in_maps = shard_inputs(x, gate_w, base_w, base_b, lora_A, lora_B)
    res = run_sharded(in_maps)
    return gather_outputs(res.results)


# revision 7
# speedup vs baseline: 1.0101x; 1.0101x over previous
"""Trainium2 Bass kernel for a LoRA-MoE layer (gate top-2 softmax routing +
dense base linear + per-expert low-rank adapters), SPMD across 8 NeuronCores.

Math (per token t):
    logits = x @ gate_w.T                      # [E]
    top-2 softmax over logits -> dense w[E] (0 for non-selected)
    out = x @ base_w.T + base_b
        + SCALING * sum_e w[e] * (x @ lora_A[e].T) @ lora_B[e].T

Key identities:
  * w folded into rank-space activations: lora_out = (low * w_rep) @ B_all.T
    with low = x @ A_all.T (A_all: [E*R, D]) -> whole MoE-LoRA is two dense
    matmuls + tiny gating vector math.
  * top-2 softmax via sigmoid: w_e = [l_e >= m2] * sigmoid(2*l_e - m1 - m2)
    (for the top-1 expert this is sigmoid(l1-l2), for top-2 sigmoid(l2-l1)).

Sharding: 8-way data parallel over tokens (T=512 tokens per core), base W
replicated and streamed.  This halves the x-load + phase-A serial head vs a
token x out-feature split; W streaming needs only ~150 GB/s per core.

Performance structure:
  * all matmul operands bf16 (host cast, free) -> PE rate unchanged, HBM
    bytes halved.
  * DMA order: adapters + x chunks first on both rings, W strictly behind x.
  * ~16 dummy matmuls at t~4us keep the PE HAM clock-gate warm before the
    first x chunk lands.
  * single shared 8-slot PSUM pool; out-tile k-loops run back-to-back while
    the gating vector chain (DVE/ACT/GPSIMD) hides behind them; each
    out-tile's B-adapter "stop" matmul is deferred two groups.
"""

import numpy as np
import ml_dtypes

import concourse.bass as bass
import concourse.bass_isa as bass_isa
import concourse.mybir as mybir
import concourse.tile as tile
from concourse import bacc
from concourse.bass_utils import run_bass_kernel_spmd

F32 = mybir.dt.float32
BF16 = mybir.dt.bfloat16
NPBF16 = ml_dtypes.bfloat16

# Problem constants
B, S, D, O = 2, 2048, 4096, 4096
E, R = 8, 16
ER = E * R  # 128
SCALING = 32.0 / 16.0

# Sharding: 8 token groups, W replicated
N_CORES = 8
TG = 8
T = (B * S) // TG       # 512 tokens per core
KT = D // 128           # 32 contraction tiles
OTN = O // 128          # 32 out tiles per core
XC = 8                  # x DMA chunks
KPC = KT // XC          # 4 k-tiles per chunk
NWARM = 8              # PE warm-up matmuls


def build_body(nc, tc, tensors):
    xT, wT, aT, gT, bT, bias2, Rm, out = tensors
    OP = mybir.AluOpType
    ACT = mybir.ActivationFunctionType

    with (
        tc.tile_pool(name="xp", bufs=XC) as xp,
        tc.tile_pool(name="wp", bufs=8) as wp,
        tc.tile_pool(name="cst", bufs=1) as cst,
        tc.tile_pool(name="gw", bufs=1) as gw,
        tc.tile_pool(name="outp", bufs=2) as outp,
        tc.tile_pool(name="ps", bufs=8, space="PSUM") as ps,
    ):
        # ---- DMA program.  The gate/lora-A/x slices for each group of 4
        #      k-tiles are fused host-side into one contiguous per-partition
        #      payload (~5KB lines, no small packets); chunks alternate
        #      across the sync and scalar rings.  W queues strictly behind
        #      the chunks on the sync ring; bias/Rm/bT slot in between the
        #      first W tiles (needed only from ~40us). ----
        GW, AW, XW = KPC * E, KPC * ER, KPC * T
        CW = GW + AW + XW
        x_tiles = []
        for c in range(XC):
            eng = nc.sync if c % 2 == 0 else nc.scalar
            xc_t = xp.tile([128, CW], BF16, tag="x", name=f"x{c}")
            eng.dma_start(out=xc_t[:], in_=xT[:, c, :])
            x_tiles.append(xc_t)

        w_tiles = []
        for ot in range(OTN):
            wv = wp.tile([128, KT * 128], BF16, tag="w", name=f"w{ot}")
            nc.sync.dma_start(out=wv[:], in_=wT[:, ot, :])
            w_tiles.append(wv)
            if ot == 3:
                bias_sb = cst.tile([128, OTN], F32)
                nc.sync.dma_start(out=bias_sb[:], in_=bias2[:])
                Rm_sb = cst.tile([E, ER], BF16)
                nc.sync.dma_start(out=Rm_sb[:], in_=Rm[:])
                bT_sb = cst.tile([ER, O], BF16)
                nc.sync.dma_start(out=bT_sb[:], in_=bT[:])

        def gs(k):
            """gate_w.T slice [128, E] for k-tile k."""
            return x_tiles[k // KPC][:, (k % KPC) * E:(k % KPC + 1) * E]

        def as_(k):
            """lora_A.T slice [128, ER] for k-tile k."""
            return x_tiles[k // KPC][:, GW + (k % KPC) * ER:
                                     GW + (k % KPC + 1) * ER]

        def xs(k):
            """x.T slice [128, T] for k-tile k."""
            return x_tiles[k // KPC][:, GW + AW + (k % KPC) * T:
                                     GW + AW + (k % KPC + 1) * T]

        # ---- PE warm-up: dummy matmuls on zeros so the HAM clock gate is
        #      already at 8/8 when the first x chunk lands.  They write the
        #      gate PSUM bank; the real k=0 matmul's start=True wipes them. ----
        warm = cst.tile([128, T], BF16)
        nc.vector.memset(warm[:], 0.0)

        gate_ps = ps.tile([E, T], F32, tag="ps", name="gateps")
        low_ps = ps.tile([ER, T], F32, tag="ps", name="lowps")
        for i in range(NWARM):
            nc.tensor.matmul(gate_ps[:], lhsT=warm[:, :E], rhs=warm[:],
                             start=True, stop=True, skip_group_check=True)

        # ---- phase A: low.T = A_all.T^T @ x.T ; gate.T = g^T @ x.T ----
        for k in range(KT):
            nc.tensor.matmul(gate_ps[:], lhsT=gs(k),
                             rhs=xs(k), start=(k == 0), stop=(k == KT - 1),
                             skip_group_check=(k == 0))
            nc.tensor.matmul(low_ps[:], lhsT=as_(k),
                             rhs=xs(k), start=(k == 0), stop=(k == KT - 1))

        # ---- gating math in [E, t] layout (DVE/ACT/GPSIMD; overlaps the
        #      first base-W matmul groups on the PE) ----
        # w_e = [l_e >= m2] * sigmoid(2*l_e - m1 - m2) * SCALING
        lowT_sb = cst.tile([ER, T], BF16, tag="lowT")
        g_sb = gw.tile([E, T], F32, tag="g")
        nc.scalar.copy(g_sb[:], gate_ps[:])
        m1 = gw.tile([E, T], F32, tag="m1")
        nc.gpsimd.partition_all_reduce(m1[:], g_sb[:], channels=E,
                                       reduce_op=bass_isa.ReduceOp.max)
        eq = gw.tile([E, T], F32, tag="eq")
        nc.vector.tensor_tensor(eq[:], g_sb[:], m1[:], op=OP.is_equal)
        gm = gw.tile([E, T], F32, tag="gm")
        nc.vector.scalar_tensor_tensor(gm[:], in0=eq[:], scalar=-1e30,
                                       in1=g_sb[:], op0=OP.mult, op1=OP.add)
        m2 = gw.tile([E, T], F32, tag="m2")
        nc.gpsimd.partition_all_reduce(m2[:], gm[:], channels=E,
                                       reduce_op=bass_isa.ReduceOp.max)
        t1 = gw.tile([E, T], F32, tag="t1")
        nc.vector.tensor_tensor(t1[:], m1[:], m2[:], op=OP.add)
        s = gw.tile([E, T], F32, tag="s")
        nc.vector.scalar_tensor_tensor(s[:], in0=g_sb[:], scalar=2.0,
                                       in1=t1[:], op0=OP.mult, op1=OP.subtract)
        sig = gw.tile([E, T], F32, tag="sig")
        nc.scalar.activation(sig[:], s[:], ACT.Sigmoid)
        mask = gw.tile([E, T], F32, tag="mask")
        nc.vector.tensor_tensor(mask[:], g_sb[:], m2[:], op=OP.is_ge)
        wsc = gw.tile([E, T], BF16, tag="wsc")
        nc.vector.scalar_tensor_tensor(wsc[:], in0=sig[:], scalar=SCALING,
                                       in1=mask[:], op0=OP.mult, op1=OP.mult)

        # ---- phase B: out.T[ot] = sum_k W[ot,k]^T @ x.T (+ B^T @ low_w.T) ----
        def w_group(ot, pb):
            for k in range(KT):
                nc.tensor.matmul(pb[:], lhsT=w_tiles[ot][:, k * 128:(k + 1) * 128],
                                 rhs=xs(k), start=(k == 0), stop=False)

        def stop_group(ot, pb):
            nc.tensor.matmul(pb[:], lhsT=bT_sb[:, ot * 128:(ot + 1) * 128],
                             rhs=lowT_sb[:], start=False, stop=True)
            o_sb = outp.tile([128, T], BF16, tag="o", name=f"o{ot}")
            nc.vector.tensor_scalar(o_sb[:], pb[:],
                                    scalar1=bias_sb[:, ot:ot + 1],
                                    scalar2=None, op0=OP.add)
            nc.gpsimd.dma_start(out=out[:, ot, :], in_=o_sb[:])

        pbs = []
        for ot in range(OTN):
            pb = ps.tile([128, T], F32, tag="ps", name=f"pb{ot}")
            pbs.append(pb)
            w_group(ot, pb)
            if ot == 1:
                # replicate each expert weight over its 16 ranks via a tiny
                # matmul, then fold into the rank-space activations.  Emitted
                # after ot1's W MMs so the PE never waits on the gating chain.
                wrep_ps = ps.tile([ER, T], F32, tag="ps", name="wrep")
                nc.tensor.matmul(wrep_ps[:], lhsT=Rm_sb[:], rhs=wsc[:],
                                 start=True, stop=True)
                wrep_sb = gw.tile([ER, T], F32, tag="wrepsb")
                nc.scalar.copy(wrep_sb[:], wrep_ps[:])
                # low_w.T = low.T * w_rep (DVE: one PSUM operand only)
                nc.vector.tensor_tensor(lowT_sb[:], low_ps[:], wrep_sb[:],
                                        op=OP.mult)
            if ot >= 2:
                stop_group(ot - 2, pbs[ot - 2])
        stop_group(OTN - 2, pbs[OTN - 2])
        stop_group(OTN - 1, pbs[OTN - 1])


def build_module(debug=False):
    nc = bacc.Bacc("TRN2", target_bir_lowering=False, debug=debug)
    CW = KPC * (E + ER + T)
    xT = nc.dram_tensor("xT", [128, XC, CW], BF16, kind="ExternalInput")
    wT = nc.dram_tensor("wT", [128, OTN, KT * 128], BF16, kind="ExternalInput")
    aT = None
    gT = None
    bT = nc.dram_tensor("bT", [ER, O], BF16, kind="ExternalInput")
    bias2 = nc.dram_tensor("bias2", [128, OTN], F32, kind="ExternalInput")
    Rm = nc.dram_tensor("Rm", [E, ER], BF16, kind="ExternalInput")
    out = nc.dram_tensor("out", [128, OTN, T], BF16, kind="ExternalOutput")
    with tile.TileContext(nc) as tc:
        build_body(nc, tc, (xT, wT, aT, gT, bT, bias2, Rm, out))
    nc.compile()
    return nc


def shard_inputs(x, gate_w, base_w, base_b, lora_A, lora_B):
    """FULL inputs -> list of 8 per-core input maps (host-side, free)."""
    x = np.asarray(x, dtype=np.float32)
    gate_w = np.asarray(gate_w, dtype=np.float32)
    base_w = np.asarray(base_w, dtype=np.float32)
    base_b = np.asarray(base_b, dtype=np.float32)
    lora_A = np.asarray(lora_A, dtype=np.float32)
    lora_B = np.asarray(lora_B, dtype=np.float32)

    xf = x.reshape(B * S, D)
    # replicated tensors; gate/lora-A slices fused into each x chunk payload
    gp = gate_w.T.reshape(XC, KPC, 128, E).transpose(2, 0, 1, 3)  # [128,XC,KPC,E]
    A_flat = lora_A.reshape(ER, D)
    ap = A_flat.T.reshape(XC, KPC, 128, ER).transpose(2, 0, 1, 3)
    B_flat = lora_B.transpose(0, 2, 1).reshape(ER, O)   # [er, o]
    bT = np.ascontiguousarray(B_flat).astype(NPBF16)
    Rm = np.repeat(np.eye(E, dtype=np.float32), R, axis=1).astype(NPBF16)
    wT = np.ascontiguousarray(
        base_w.reshape(OTN, 128, KT, 128).transpose(3, 0, 2, 1)
        .reshape(128, OTN, KT * 128)).astype(NPBF16)
    bias2 = np.ascontiguousarray(base_b.reshape(OTN, 128).T)
    gac = np.concatenate([
        gp.reshape(128, XC, KPC * E),
        ap.reshape(128, XC, KPC * ER)], axis=2)          # [128, XC, GW+AW]

    in_maps = []
    for c in range(N_CORES):
        x_c = xf[c * T:(c + 1) * T]                         # [T, D]
        xp_ = x_c.T.reshape(XC, KPC, 128, T).transpose(2, 0, 1, 3)
        xT = np.ascontiguousarray(np.concatenate(
            [gac, xp_.reshape(128, XC, KPC * T)], axis=2)).astype(NPBF16)
        in_maps.append({"xT": xT, "wT": wT,
                        "bT": bT, "bias2": bias2, "Rm": Rm})
    return in_maps


def gather_outputs(results):
    """list of 8 per-core result maps -> FULL output [B, S, O]."""
    full = np.empty((B * S, O), dtype=np.float32)
    for c in range(N_CORES):
        oc = np.asarray(results[c]["out"], dtype=np.float32)  # [128, OTN, T]
        full[c * T:(c + 1) * T, :] = oc.transpose(2, 1, 0).reshape(T, O)
    return full.reshape(B, S, O)


_NC_CACHE = {}


def _get_module():
    if "nc" not in _NC_CACHE:
        _NC_CACHE["nc"] = build_module()
    return _NC_CACHE["nc"]


def run_sharded(in_maps, **run_kwargs):
    nc = _get_module()
    return run_bass_kernel_spmd(nc, in_maps, list(range(N_CORES)), **run_kwargs)


def kernel(x, gate_w, base_w, base_b, lora_A, lora_B):
    in_maps = shard_inputs(x, gate_w, base_w, base_b, lora_A, lora_B)
    res = run_sharded(in_maps)
    return gather_outputs(res.results)
